# revision 20
# baseline (speedup 1.0000x reference)
"""Multi-head attention Trainium2 kernel (B=4, S=2048, D=1024, H=16, A=64).

Sharding: 8 cores = batch (4) x head-half (2). Core i handles batch i//2,
heads (i%2)*8 .. (i%2)*8+8. No collectives; host assembles output.

Key design points (v2 — fp16 single-pass matmuls):
  - ALL matmuls run in 16-bit dtypes (1 cyc/row single-pass on the PE at
    2.4 GHz). The previous f32r version was silently lowered to two HW
    passes (fp32_mode=HIGH ~2cyc/row + LOW_HIGH 1cyc/row), doubling PE time.
  - q/k/v arrive HOST-pretransposed [D, S] as fp16 (11-bit mantissa:
    score error sigma ~0.002 absolute, negligible through softmax).
    Projections fp16 x fp16 -> f32 PSUM -> qhT/khT stored fp16.
  - scores^T [Sk, Sq] per head-pair via row-packed K=64 matmul pairs
    (tile_position (0,0)/(64,0)) into one [128,1024] PSUM tile -> single
    1024-col exp on ACT (no max subtraction: |s|max ~48 << 88) with BF16
    output (bf16 shares f32's exponent range; exp(s) can reach e^+48).
  - attn' [65, Sq]: lhsT = vh bf16 [Sk,65] (64 v-cols + ones column),
    rhs = wt bf16. Row 64 accumulates the softmax denominator.
  - NO on-chip normalization: the [65,512] numerator+denominator block is
    copied PSUM->SBUF (DVE) and DMA'd out raw; the host does num/den.
  - Projections are emitted per-pair BETWEEN attention blocks; the Tile
    dataflow scheduler runs them in PE gaps while ACT (exp) is the
    per-block bottleneck, so pair p+1's projections hide under pair p's
    ACT-bound attention. Prologue = proj(pair 0) only.
"""

import sys

sys.path.insert(0, "/opt/trn_rl_repo")

import numpy as np

B, S, D = 4, 2048, 1024
H, A = 16, 64
NCORES = 8
HL = H // 2          # heads per core
NPAIR = HL // 2      # head pairs per core
ND = D // 128        # D chunks
NSQ = S // 512       # Sq chunks
NSK = S // 128       # Sk tiles
AC = A + 1           # vh columns incl. ones column


def _build():
    import concourse.tile as tile
    from concourse import bacc, mybir

    F32 = mybir.dt.float32
    F16 = mybir.dt.float16
    BF16 = mybir.dt.bfloat16
    ADD = mybir.AluOpType.add
    EXP = mybir.ActivationFunctionType.Exp

    nc = bacc.Bacc("TRN2")

    x_d = {}
    for x in ("v", "k", "q"):
        x_d[x] = nc.dram_tensor(f"x{x}", [D, S], F16, kind="ExternalInput").ap()
    wq_d = nc.dram_tensor("wq", [D, HL * A], F16, kind="ExternalInput").ap()
    wk_d = nc.dram_tensor("wk", [D, HL * A], F16, kind="ExternalInput").ap()
    wv_d = nc.dram_tensor("wv", [D, HL * AC], F16, kind="ExternalInput").ap()
    bq_d = nc.dram_tensor("bq", [128, NPAIR], F32, kind="ExternalInput").ap()
    bk_d = nc.dram_tensor("bk", [128, NPAIR], F32, kind="ExternalInput").ap()
    bv_d = nc.dram_tensor("bv", [1, HL * AC], F16, kind="ExternalInput").ap()
    on_d = nc.dram_tensor("ones1", [1, 128], F16, kind="ExternalInput").ap()
    # per (pair, head-in-pair, sq): [65, 512] = numerator rows 0-63, den row 64
    out_d = nc.dram_tensor(
        "out", [NPAIR * 2 * NSQ * AC, 512], F32, kind="ExternalOutput"
    ).ap()

    with tile.TileContext(nc) as tc:
        with (
            tc.tile_pool(name="consts", bufs=1) as consts,
            tc.tile_pool(name="stage", bufs=1) as stage,
            tc.tile_pool(name="persist", bufs=1) as persist,
            tc.tile_pool(name="work", bufs=1) as work,
            tc.tile_pool(name="ps", bufs=1, space="PSUM") as ps,
        ):
            ones1 = consts.tile([1, 128], F16, tag="ones1")
            bq_sb = consts.tile([128, NPAIR], F32, tag="bq")
            bk_sb = consts.tile([128, NPAIR], F32, tag="bk")
            bv_sb = consts.tile([1, HL * AC], F16, tag="bv")
            nc.sync.dma_start(ones1, on_d)
            nc.sync.dma_start(bq_sb, bq_d)
            nc.sync.dma_start(bk_sb, bk_d)
            nc.sync.dma_start(bv_sb, bv_d)

            # weights (k/q first: the exp-critical path is kproj->qproj->scores)
            wk_sb = stage.tile([128, ND, HL * A], F16, tag="wk", name="wk_sb")
            nc.sync.dma_start(wk_sb, wk_d.rearrange("(c p) n -> p c n", p=128))
            wq_sb = stage.tile([128, ND, HL * A], F16, tag="wq", name="wq_sb")
            nc.sync.dma_start(wq_sb, wq_d.rearrange("(c p) n -> p c n", p=128))
            wv_sb = stage.tile([128, ND, HL * AC], F16, tag="wv", name="wv_sb")
            nc.sync.dma_start(wv_sb, wv_d.rearrange("(c p) n -> p c n", p=128))

            # full [D, S] input staging, one [128, S] tile per D-chunk.
            # The first k/q chunks are 512-col (small, land fast) so
            # kproj(0,h0)/qproj(0,h0) — the exp-critical path — can start
            # ~4-6us in; later chunks are 1024-col for DMA-line efficiency.
            xt = {}
            for x in ("v", "k", "q"):
                xt[x] = [
                    stage.tile([128, S], F16, tag=f"x{x}{d}", name=f"x{x}{d}")
                    for d in range(ND)
                ]
            dma_plan = (
                ("k", 0, 512), ("q", 0, 512), ("k", 512, 512),
                ("v", 0, 1024), ("q", 512, 512), ("k", 1024, 1024),
                ("v", 1024, 1024), ("q", 1024, 1024),
            )
            for x, c0, w in dma_plan:
                for d in range(ND):
                    nc.sync.dma_start(
                        xt[x][d][:, c0 : c0 + w],
                        x_d[x][d * 128 : (d + 1) * 128, c0 : c0 + w],
                    )

            qhT = [
                persist.tile([128, S], F16, tag=f"qhT{p}", name=f"qhT{p}")
                for p in range(NPAIR)
            ]
            khT = [
                persist.tile([128, S], F16, tag=f"khT{p}", name=f"khT{p}")
                for p in range(NPAIR)
            ]
            vh = persist.tile([128, HL, NSK, AC], BF16, tag="vh")

            def vproj(p, m0=0, m1=NSK):
                # vh[:, 2p:2p+2, m, :]: [Sk, 2*65] with ones column via
                # K=1 bias matmul (weights col 64 of each head = 0, bias = 1)
                cw = 2 * AC
                for m in range(m0, m1):
                    pv = ps.tile([128, cw], F32, tag="pp", name="pv", bufs=2)
                    for d in range(ND):
                        nc.tensor.matmul(
                            pv,
                            xt["v"][d][:, m * 128 : (m + 1) * 128],
                            wv_sb[:, d, p * cw : (p + 1) * cw],
                            start=(d == 0),
                            stop=False,
                        )
                    nc.tensor.matmul(
                        pv, ones1, bv_sb[:, p * cw : (p + 1) * cw],
                        start=False, stop=True,
                    )
                    nc.vector.tensor_copy(
                        vh[:, 2 * p : 2 * p + 2, m, :],
                        pv.rearrange("p (h c) -> p h c", h=2),
                    )

            def kqproj(x, p, w_sb, bias_sb, dst, half):
                # dst[p] [128 (2 heads), S] fp16 = W_pair^T @ x^T + bias
                col = half * 512
                pph = ps.tile([128, 512], F32, tag="pp", name="pph", bufs=2)
                for d in range(ND):
                    nc.tensor.matmul(
                        pph,
                        w_sb[:, d, p * 128 : (p + 1) * 128],
                        xt[x][d][:, col : col + 512],
                        start=(d == 0),
                        stop=(d == ND - 1),
                    )
                nc.vector.tensor_scalar(
                    dst[p][:, col : col + 512],
                    pph,
                    bias_sb[:, p : p + 1],
                    None,
                    ADD,
                )

            # Global software pipeline across ALL (p, sq, sk): scores/exp
            # lead attention by 2 steps and the pipeline never drains at
            # block boundaries, so the exp stream on ACT stays contiguous.
            Pstate = {}

            def emit_scores(p, sq, sk):
                Sc = ps.tile([128, 1024], F32, tag="sc", name="Sc", bufs=2)
                nc.tensor.matmul(
                    Sc[:, 0:512],
                    khT[p][0:64, sk * 128 : (sk + 1) * 128],
                    qhT[p][0:64, sq * 512 : (sq + 1) * 512],
                    start=True,
                    stop=True,
                    tile_position=(0, 0),
                )
                nc.tensor.matmul(
                    Sc[:, 512:1024],
                    khT[p][64:128, sk * 128 : (sk + 1) * 128],
                    qhT[p][64:128, sq * 512 : (sq + 1) * 512],
                    start=True,
                    stop=True,
                    tile_position=(64, 0),
                )
                wt = work.tile([128, 1024], BF16, tag="wt", name="wt", bufs=4)
                nc.scalar.activation(wt, Sc, EXP)
                return wt

            def emit_attn(p, sq, k0, wt):
                h0, h1 = 2 * p, 2 * p + 1
                if k0 == 0:
                    P0 = ps.tile([AC, 512], F32, tag="att", name="P0", bufs=2)
                    P1 = ps.tile([AC, 512], F32, tag="att", name="P1", bufs=2)
                    Pstate[(p, sq)] = (P0, P1)
                P0, P1 = Pstate[(p, sq)]
                st = k0 == 0
                sp = k0 == NSK - 1
                nc.tensor.matmul(
                    P0, vh[:, h0, k0, :], wt[:, 0:512], start=st, stop=sp,
                )
                nc.tensor.matmul(
                    P1, vh[:, h1, k0, :], wt[:, 512:1024], start=st, stop=sp,
                )
                if sp:
                    for hh, P in ((0, P0), (1, P1)):
                        atts = work.tile(
                            [AC, 512], F32, tag="atts", name="atts", bufs=4
                        )
                        nc.vector.tensor_copy(atts, P)
                        blk = (p * 2 + hh) * NSQ + sq
                        nc.sync.dma_start(
                            out_d[blk * AC : (blk + 1) * AC, :], atts
                        )
                    del Pstate[(p, sq)]

            # Emission order == scheduler priority, and dep tracking is
            # program-order-causal, so each projection chunk must be emitted
            # before its first reader but as CLOSE to it as possible (earlier
            # emission = higher priority = lumps at pair boundaries). The
            # fill lists below spread next-pair projections through the
            # current pair's ACT-bound blocks, meeting every deadline:
            #   kp(p,0/1), qp(p,0) before block(p,0); kp(p,2/3) before its
            #   sk=8 scores; qp(p,h) before block(p,h); v(p,m) before the
            #   attn read at sk=m+2 of block(p,0).
            def kp(p, h):
                return lambda: kqproj("k", p, wk_sb, bk_sb, khT, h)

            def qp(p, h):
                return lambda: kqproj("q", p, wq_sb, bq_sb, qhT, h)

            def vp(p, m):
                return lambda: vproj(p, m, m + 1)

            kp(0, 0)()
            qp(0, 0)()
            kp(0, 1)()
            vproj(0)

            # fill items: (closure, cost_ns, deadline_sk or None). Deadlines
            # force emission before the reading instruction (program-order-
            # causal deps); pacing (one chunk per ~KQ_COST of accumulated
            # slack) keeps proj from starving the exp stream in bursts.
            KQ, VC = 1750, 500
            fills = {
                (0, 0): [(kp(0, 2), KQ, 8), (kp(0, 3), KQ, 12), (qp(0, 1), KQ, 15)],
                (0, 1): [(qp(0, 2), KQ, None), (kp(1, 0), KQ, None)]
                + [(vp(1, m), VC, None) for m in range(3)],
                (0, 2): [(qp(0, 3), KQ, None), (kp(1, 1), KQ, None)]
                + [(vp(1, m), VC, None) for m in range(3, 6)],
                (0, 3): [(qp(1, 0), KQ, None), (kp(1, 2), KQ, None),
                         (qp(1, 1), KQ, None)]
                + [(vp(1, m), VC, None) for m in range(6, 13)],
                (1, 0): [(kp(1, 3), KQ, 12)]
                + [(vp(1, m), VC, m + 2) for m in range(13, 16)]
                + [(qp(1, 2), KQ, None)],
                (1, 1): [(qp(1, 3), KQ, None), (kp(2, 0), KQ, None)]
                + [(vp(2, m), VC, None) for m in range(3)],
                (1, 2): [(qp(2, 0), KQ, None), (kp(2, 1), KQ, None)]
                + [(vp(2, m), VC, None) for m in range(3, 6)],
                (1, 3): [(kp(2, 2), KQ, None), (qp(2, 1), KQ, None)]
                + [(vp(2, m), VC, None) for m in range(6, 13)],
                (2, 0): [(kp(2, 3), KQ, 12)]
                + [(vp(2, m), VC, m + 2) for m in range(13, 16)]
                + [(qp(2, 2), KQ, None)],
                (2, 1): [(qp(2, 3), KQ, None), (kp(3, 0), KQ, None)]
                + [(vp(3, m), VC, None) for m in range(3)],
                (2, 2): [(qp(3, 0), KQ, None), (kp(3, 1), KQ, None)]
                + [(vp(3, m), VC, None) for m in range(3, 6)],
                (2, 3): [(kp(3, 2), KQ, None), (qp(3, 1), KQ, None)]
                + [(vp(3, m), VC, None) for m in range(6, 13)],
                (3, 0): [(kp(3, 3), KQ, 12)]
                + [(vp(3, m), VC, m + 2) for m in range(13, 16)]
                + [(qp(3, 2), KQ, None)],
                (3, 1): [(qp(3, 3), KQ, None)],
            }
            SLACK = 190  # ns of PE slack per sk under the ACT-bound stream
            debt = 0
            stream = [
                (p, sq, sk)
                for p in range(NPAIR)
                for sq in range(NSQ)
                for sk in range(NSK)
            ]
            pend = []
            for p, sq, sk in stream:
                fl = fills.get((p, sq))
                if fl and sk >= 2:
                    while fl and fl[0][2] is not None and fl[0][2] <= sk:
                        c = fl.pop(0)
                        c[0]()
                        debt += c[1]
                    if fl and (debt <= 0 or sk == NSK - 1):
                        # pace by accumulated slack; force-drain the whole
                        # list at the block's last slot (items must precede
                        # their readers in later blocks)
                        while fl:
                            c = fl.pop(0)
                            c[0]()
                            debt += c[1]
                            if sk < NSK - 1:
                                break
                debt = max(debt - SLACK, -SLACK)
                wt = emit_scores(p, sq, sk)
                pend.append((p, sq, sk, wt))
                if len(pend) > 2:
                    emit_attn(*pend.pop(0))
            while pend:
                emit_attn(*pend.pop(0))

    nc.compile()
    return nc


_NC_CACHE = None
_LAST_IN_MAPS = None


def kernel(**inputs: np.ndarray) -> np.ndarray:
    global _NC_CACHE, _LAST_IN_MAPS

    from concourse.bass_utils import run_bass_kernel_spmd

    q = np.ascontiguousarray(inputs["q"], dtype=np.float32)
    k = np.ascontiguousarray(inputs["k"], dtype=np.float32)
    v = np.ascontiguousarray(inputs["v"], dtype=np.float32)
    Wq = np.asarray(inputs["Wq"], dtype=np.float32)
    Wk = np.asarray(inputs["Wk"], dtype=np.float32)
    Wv = np.asarray(inputs["Wv"], dtype=np.float32)
    bq = np.asarray(inputs["bq"], dtype=np.float32)
    bk = np.asarray(inputs["bk"], dtype=np.float32)
    bv = np.asarray(inputs["bv"], dtype=np.float32)

    if _NC_CACHE is None:
        _NC_CACHE = _build()
    nc = _NC_CACHE

    ones1 = np.ones((1, 128), dtype=np.float16)

    def pack_w(W, g):
        # [H,D,A] slice -> [D, HL*A], heads side by side
        return np.ascontiguousarray(
            W[g * HL : (g + 1) * HL].transpose(1, 0, 2).reshape(D, HL * A)
        ).astype(np.float16)

    def pack_wv(W, bvv, g):
        # augmented: per head 65 columns (64 weights + zero col); bias row gets 1.0
        Wg = W[g * HL : (g + 1) * HL]  # [HL, D, A]
        Wa = np.zeros((HL, D, AC), dtype=np.float32)
        Wa[:, :, :A] = Wg
        ba = np.zeros((1, HL * AC), dtype=np.float32)
        bb = bvv[g * HL : (g + 1) * HL]  # [HL, A]
        for h in range(HL):
            ba[0, h * AC : h * AC + A] = bb[h]
            ba[0, h * AC + A] = 1.0
        return (
            np.ascontiguousarray(
                Wa.transpose(1, 0, 2).reshape(D, HL * AC)
            ).astype(np.float16),
            ba.astype(np.float16),
        )

    def pack_b(bvec, g):
        # [H,A] slice -> [128, NPAIR]: column p = concat(b[2p], b[2p+1])
        bg = bvec[g * HL : (g + 1) * HL]
        return np.ascontiguousarray(bg.reshape(NPAIR, 128).T)

    xT_cache = {}
    for b_ in range(B):
        xT_cache[b_] = {
            "q": np.ascontiguousarray(q[b_].T).astype(np.float16),
            "k": np.ascontiguousarray(k[b_].T).astype(np.float16),
            "v": np.ascontiguousarray(v[b_].T).astype(np.float16),
        }

    in_maps = []
    for i in range(NCORES):
        b_, g = i // 2, i % 2
        wv_p, bv_p = pack_wv(Wv, bv, g)
        xc = xT_cache[b_]
        in_maps.append(
            {
                "xq": xc["q"],
                "xk": xc["k"],
                "xv": xc["v"],
                "wq": pack_w(Wq, g),
                "wk": pack_w(Wk, g),
                "wv": wv_p,
                "bq": pack_b(bq, g),
                "bk": pack_b(bk, g),
                "bv": bv_p,
                "ones1": ones1,
            }
        )

    _LAST_IN_MAPS = in_maps
    res = run_bass_kernel_spmd(nc, in_maps, core_ids=list(range(NCORES)))

    out = np.empty((B, S, H * A), dtype=np.float32)
    for i in range(NCORES):
        b_, g = i // 2, i % 2
        r = res.results[i]["out"].reshape(NPAIR, 2, NSQ, AC, 512)
        num = r[:, :, :, 0:A, :]  # [p, hh, sq, a, q']
        den = r[:, :, :, A : A + 1, :]
        y = num / den  # [p, hh, sq, a, q']
        # -> [sq, q', p, hh, a] -> [S, HL*A]
        y = y.transpose(2, 4, 0, 1, 3).reshape(S, HL * A)
        out[b_, :, g * HL * A : (g + 1) * HL * A] = y
    return out


# revision 23
# speedup vs baseline: 1.0132x; 1.0132x over previous
"""Multi-head attention Trainium2 kernel (B=4, S=2048, D=1024, H=16, A=64).

Sharding: 8 cores = batch (4) x head-half (2). Core i handles batch i//2,
heads (i%2)*8 .. (i%2)*8+8. No collectives; host assembles output.

Key design points (v2 — fp16 single-pass matmuls):
  - ALL matmuls run in 16-bit dtypes (1 cyc/row single-pass on the PE at
    2.4 GHz). The previous f32r version was silently lowered to two HW
    passes (fp32_mode=HIGH ~2cyc/row + LOW_HIGH 1cyc/row), doubling PE time.
  - q/k/v arrive HOST-pretransposed [D, S] as fp16 (11-bit mantissa:
    score error sigma ~0.002 absolute, negligible through softmax).
    Projections fp16 x fp16 -> f32 PSUM -> qhT/khT stored fp16.
  - scores^T [Sk, Sq] per head-pair via row-packed K=64 matmul pairs
    (tile_position (0,0)/(64,0)) into one [128,1024] PSUM tile -> single
    1024-col exp on ACT (no max subtraction: |s|max ~48 << 88) with BF16
    output (bf16 shares f32's exponent range; exp(s) can reach e^+48).
  - attn' [65, Sq]: lhsT = vh bf16 [Sk,65] (64 v-cols + ones column),
    rhs = wt bf16. Row 64 accumulates the softmax denominator.
  - NO on-chip normalization: the [65,512] numerator+denominator block is
    copied PSUM->SBUF (DVE) and DMA'd out raw; the host does num/den.
  - Projections are emitted per-pair BETWEEN attention blocks; the Tile
    dataflow scheduler runs them in PE gaps while ACT (exp) is the
    per-block bottleneck, so pair p+1's projections hide under pair p's
    ACT-bound attention. Prologue = proj(pair 0) only.
"""

import sys

sys.path.insert(0, "/opt/trn_rl_repo")

import numpy as np

B, S, D = 4, 2048, 1024
H, A = 16, 64
NCORES = 8
HL = H // 2          # heads per core
NPAIR = HL // 2      # head pairs per core
ND = D // 128        # D chunks
NSQ = S // 512       # Sq chunks
NSK = S // 128       # Sk tiles
AC = A + 1           # vh columns incl. ones column


def _build():
    import concourse.tile as tile
    from concourse import bacc, mybir

    F32 = mybir.dt.float32
    F16 = mybir.dt.float16
    BF16 = mybir.dt.bfloat16
    ADD = mybir.AluOpType.add
    EXP = mybir.ActivationFunctionType.Exp

    nc = bacc.Bacc("TRN2")

    x_d = {}
    for x in ("v", "k", "q"):
        x_d[x] = nc.dram_tensor(f"x{x}", [D, S], F16, kind="ExternalInput").ap()
    wq_d = nc.dram_tensor("wq", [D, HL * A], F16, kind="ExternalInput").ap()
    wk_d = nc.dram_tensor("wk", [D, HL * A], F16, kind="ExternalInput").ap()
    wv_d = nc.dram_tensor("wv", [D, HL * AC], F16, kind="ExternalInput").ap()
    bq_d = nc.dram_tensor("bq", [128, NPAIR], F32, kind="ExternalInput").ap()
    bk_d = nc.dram_tensor("bk", [128, NPAIR], F32, kind="ExternalInput").ap()
    bv_d = nc.dram_tensor("bv", [1, HL * AC], F16, kind="ExternalInput").ap()
    on_d = nc.dram_tensor("ones1", [1, 128], F16, kind="ExternalInput").ap()
    # per (pair, head-in-pair, sq): [65, 512] = numerator rows 0-63, den row 64
    out_d = nc.dram_tensor(
        "out", [NPAIR * 2 * NSQ * AC, 512], F32, kind="ExternalOutput"
    ).ap()

    with tile.TileContext(nc) as tc:
        with (
            tc.tile_pool(name="consts", bufs=1) as consts,
            tc.tile_pool(name="stage", bufs=1) as stage,
            tc.tile_pool(name="persist", bufs=1) as persist,
            tc.tile_pool(name="work", bufs=1) as work,
            tc.tile_pool(name="ps", bufs=1, space="PSUM") as ps,
        ):
            ones1 = consts.tile([1, 128], F16, tag="ones1")
            bq_sb = consts.tile([128, NPAIR], F32, tag="bq")
            bk_sb = consts.tile([128, NPAIR], F32, tag="bk")
            bv_sb = consts.tile([1, HL * AC], F16, tag="bv")
            nc.sync.dma_start(ones1, on_d)
            nc.sync.dma_start(bq_sb, bq_d)
            nc.sync.dma_start(bk_sb, bk_d)
            nc.sync.dma_start(bv_sb, bv_d)

            # Input staging: one [128, S] tile per D-chunk. DMA bandwidth is
            # serialized (~400GB/s aggregate), so order = arrival time.
            # Weights go just-in-time before their first consumer; the first
            # k/q chunks are 512-col so kproj(0,h0)/qproj(0,h0) — the
            # exp-critical path — start as early as possible.
            xt = {}
            for x in ("v", "k", "q"):
                xt[x] = [
                    stage.tile([128, S], F16, tag=f"x{x}{d}", name=f"x{x}{d}")
                    for d in range(ND)
                ]
            wk_sb = stage.tile([128, ND, HL * A], F16, tag="wk", name="wk_sb")
            wq_sb = stage.tile([128, ND, HL * A], F16, tag="wq", name="wq_sb")
            wv_sb = stage.tile([128, ND, HL * AC], F16, tag="wv", name="wv_sb")

            def xdma(x, c0, w):
                for d in range(ND):
                    nc.sync.dma_start(
                        xt[x][d][:, c0 : c0 + w],
                        x_d[x][d * 128 : (d + 1) * 128, c0 : c0 + w],
                    )

            nc.sync.dma_start(wk_sb, wk_d.rearrange("(c p) n -> p c n", p=128))
            xdma("k", 0, 512)
            nc.sync.dma_start(wq_sb, wq_d.rearrange("(c p) n -> p c n", p=128))
            xdma("q", 0, 512)
            xdma("k", 512, 512)
            nc.sync.dma_start(wv_sb, wv_d.rearrange("(c p) n -> p c n", p=128))
            xdma("v", 0, 1024)
            xdma("q", 512, 512)
            xdma("k", 1024, 1024)
            xdma("v", 1024, 1024)
            xdma("q", 1024, 1024)

            qhT = [
                persist.tile([128, S], F16, tag=f"qhT{p}", name=f"qhT{p}")
                for p in range(NPAIR)
            ]
            khT = [
                persist.tile([128, S], F16, tag=f"khT{p}", name=f"khT{p}")
                for p in range(NPAIR)
            ]
            vh = persist.tile([128, HL, NSK, AC], BF16, tag="vh")

            def vproj(p, m0=0, m1=NSK):
                # vh[:, 2p:2p+2, m, :]: [Sk, 2*65] with ones column via
                # K=1 bias matmul (weights col 64 of each head = 0, bias = 1)
                cw = 2 * AC
                for m in range(m0, m1):
                    pv = ps.tile([128, cw], F32, tag="pp", name="pv", bufs=2)
                    for d in range(ND):
                        nc.tensor.matmul(
                            pv,
                            xt["v"][d][:, m * 128 : (m + 1) * 128],
                            wv_sb[:, d, p * cw : (p + 1) * cw],
                            start=(d == 0),
                            stop=False,
                        )
                    nc.tensor.matmul(
                        pv, ones1, bv_sb[:, p * cw : (p + 1) * cw],
                        start=False, stop=True,
                    )
                    nc.vector.tensor_copy(
                        vh[:, 2 * p : 2 * p + 2, m, :],
                        pv.rearrange("p (h c) -> p h c", h=2),
                    )

            def kqproj(x, p, w_sb, bias_sb, dst, half):
                # dst[p] [128 (2 heads), S] fp16 = W_pair^T @ x^T + bias
                col = half * 512
                pph = ps.tile([128, 512], F32, tag="pp", name="pph", bufs=2)
                for d in range(ND):
                    nc.tensor.matmul(
                        pph,
                        w_sb[:, d, p * 128 : (p + 1) * 128],
                        xt[x][d][:, col : col + 512],
                        start=(d == 0),
                        stop=(d == ND - 1),
                    )
                nc.vector.tensor_scalar(
                    dst[p][:, col : col + 512],
                    pph,
                    bias_sb[:, p : p + 1],
                    None,
                    ADD,
                )

            # Global software pipeline across ALL (p, sq, sk): scores/exp
            # lead attention by 2 steps and the pipeline never drains at
            # block boundaries, so the exp stream on ACT stays contiguous.
            Pstate = {}

            def emit_scores(p, sq, sk):
                Sc = ps.tile([128, 1024], F32, tag="sc", name="Sc", bufs=2)
                nc.tensor.matmul(
                    Sc[:, 0:512],
                    khT[p][0:64, sk * 128 : (sk + 1) * 128],
                    qhT[p][0:64, sq * 512 : (sq + 1) * 512],
                    start=True,
                    stop=True,
                    tile_position=(0, 0),
                )
                nc.tensor.matmul(
                    Sc[:, 512:1024],
                    khT[p][64:128, sk * 128 : (sk + 1) * 128],
                    qhT[p][64:128, sq * 512 : (sq + 1) * 512],
                    start=True,
                    stop=True,
                    tile_position=(64, 0),
                )
                wt = work.tile([128, 1024], BF16, tag="wt", name="wt", bufs=6)
                nc.scalar.activation(wt, Sc, EXP)
                return wt

            def emit_attn(p, sq, k0, wt):
                h0, h1 = 2 * p, 2 * p + 1
                if k0 == 0:
                    P0 = ps.tile([AC, 512], F32, tag="att", name="P0", bufs=2)
                    P1 = ps.tile([AC, 512], F32, tag="att", name="P1", bufs=2)
                    Pstate[(p, sq)] = (P0, P1)
                P0, P1 = Pstate[(p, sq)]
                st = k0 == 0
                sp = k0 == NSK - 1
                nc.tensor.matmul(
                    P0, vh[:, h0, k0, :], wt[:, 0:512], start=st, stop=sp,
                )
                nc.tensor.matmul(
                    P1, vh[:, h1, k0, :], wt[:, 512:1024], start=st, stop=sp,
                )
                if sp:
                    for hh, P in ((0, P0), (1, P1)):
                        atts = work.tile(
                            [AC, 512], F32, tag="atts", name="atts", bufs=4
                        )
                        nc.vector.tensor_copy(atts, P)
                        blk = (p * 2 + hh) * NSQ + sq
                        nc.sync.dma_start(
                            out_d[blk * AC : (blk + 1) * AC, :], atts
                        )
                    del Pstate[(p, sq)]

            # Emission order == scheduler priority, and dep tracking is
            # program-order-causal, so each projection chunk must be emitted
            # before its first reader but as CLOSE to it as possible (earlier
            # emission = higher priority = lumps at pair boundaries). The
            # fill lists below spread next-pair projections through the
            # current pair's ACT-bound blocks, meeting every deadline:
            #   kp(p,0/1), qp(p,0) before block(p,0); kp(p,2/3) before its
            #   sk=8 scores; qp(p,h) before block(p,h); v(p,m) before the
            #   attn read at sk=m+2 of block(p,0).
            def kp(p, h):
                return lambda: kqproj("k", p, wk_sb, bk_sb, khT, h)

            def qp(p, h):
                return lambda: kqproj("q", p, wq_sb, bq_sb, qhT, h)

            def vp(p, m):
                return lambda: vproj(p, m, m + 1)

            kp(0, 0)()
            qp(0, 0)()
            kp(0, 1)()
            vproj(0)

            # fill items: (closure, cost_ns, deadline_sk or None). Deadlines
            # force emission before the reading instruction (program-order-
            # causal deps); pacing (one chunk per ~KQ_COST of accumulated
            # slack) keeps proj from starving the exp stream in bursts.
            KQ, VC = 1750, 500
            fills = {
                (0, 0): [(kp(0, 2), KQ, 8), (kp(0, 3), KQ, 12), (qp(0, 1), KQ, 15)],
                (0, 1): [(qp(0, 2), KQ, None), (kp(1, 0), KQ, None)]
                + [(vp(1, m), VC, None) for m in range(3)],
                (0, 2): [(qp(0, 3), KQ, None), (kp(1, 1), KQ, None)]
                + [(vp(1, m), VC, None) for m in range(3, 6)],
                (0, 3): [(qp(1, 0), KQ, None), (kp(1, 2), KQ, None),
                         (qp(1, 1), KQ, None)]
                + [(vp(1, m), VC, None) for m in range(6, 13)],
                (1, 0): [(kp(1, 3), KQ, 12)]
                + [(vp(1, m), VC, m + 2) for m in range(13, 16)]
                + [(qp(1, 2), KQ, None)],
                (1, 1): [(qp(1, 3), KQ, None), (kp(2, 0), KQ, None)]
                + [(vp(2, m), VC, None) for m in range(3)],
                (1, 2): [(qp(2, 0), KQ, None), (kp(2, 1), KQ, None)]
                + [(vp(2, m), VC, None) for m in range(3, 6)],
                (1, 3): [(kp(2, 2), KQ, None), (qp(2, 1), KQ, None)]
                + [(vp(2, m), VC, None) for m in range(6, 13)],
                (2, 0): [(kp(2, 3), KQ, 12)]
                + [(vp(2, m), VC, m + 2) for m in range(13, 16)]
                + [(qp(2, 2), KQ, None)],
                (2, 1): [(qp(2, 3), KQ, None), (kp(3, 0), KQ, None)]
                + [(vp(3, m), VC, None) for m in range(3)],
                (2, 2): [(qp(3, 0), KQ, None), (kp(3, 1), KQ, None)]
                + [(vp(3, m), VC, None) for m in range(3, 6)],
                (2, 3): [(kp(3, 2), KQ, None), (qp(3, 1), KQ, None)]
                + [(vp(3, m), VC, None) for m in range(6, 13)],
                (3, 0): [(kp(3, 3), KQ, 12)]
                + [(vp(3, m), VC, m + 2) for m in range(13, 16)]
                + [(qp(3, 2), KQ, None)],
                (3, 1): [(qp(3, 3), KQ, None)],
            }
            SLACK = 190  # ns of PE slack per sk under the ACT-bound stream
            debt = 0
            stream = [
                (p, sq, sk)
                for p in range(NPAIR)
                for sq in range(NSQ)
                for sk in range(NSK)
            ]
            pend = []
            for p, sq, sk in stream:
                fl = fills.get((p, sq))
                if fl and sk >= 2:
                    while fl and fl[0][2] is not None and fl[0][2] <= sk:
                        c = fl.pop(0)
                        c[0]()
                        debt += c[1]
                    # pace by accumulated slack; spread any leftovers over
                    # the last slots (items must precede their readers in
                    # later blocks, so nothing may survive the block)
                    leftover_push = len(fl) >= NSK - sk
                    if fl and (debt <= 0 or leftover_push):
                        while fl:
                            c = fl.pop(0)
                            c[0]()
                            debt += c[1]
                            if len(fl) < NSK - sk - 1:
                                break
                debt = max(debt - SLACK, -SLACK)
                wt = emit_scores(p, sq, sk)
                pend.append((p, sq, sk, wt))
                if len(pend) > 2:
                    emit_attn(*pend.pop(0))
            while pend:
                emit_attn(*pend.pop(0))

    nc.compile()
    return nc


_NC_CACHE = None
_LAST_IN_MAPS = None


def kernel(**inputs: np.ndarray) -> np.ndarray:
    global _NC_CACHE, _LAST_IN_MAPS

    from concourse.bass_utils import run_bass_kernel_spmd

    q = np.ascontiguousarray(inputs["q"], dtype=np.float32)
    k = np.ascontiguousarray(inputs["k"], dtype=np.float32)
    v = np.ascontiguousarray(inputs["v"], dtype=np.float32)
    Wq = np.asarray(inputs["Wq"], dtype=np.float32)
    Wk = np.asarray(inputs["Wk"], dtype=np.float32)
    Wv = np.asarray(inputs["Wv"], dtype=np.float32)
    bq = np.asarray(inputs["bq"], dtype=np.float32)
    bk = np.asarray(inputs["bk"], dtype=np.float32)
    bv = np.asarray(inputs["bv"], dtype=np.float32)

    if _NC_CACHE is None:
        _NC_CACHE = _build()
    nc = _NC_CACHE

    ones1 = np.ones((1, 128), dtype=np.float16)

    def pack_w(W, g):
        # [H,D,A] slice -> [D, HL*A], heads side by side
        return np.ascontiguousarray(
            W[g * HL : (g + 1) * HL].transpose(1, 0, 2).reshape(D, HL * A)
        ).astype(np.float16)

    def pack_wv(W, bvv, g):
        # augmented: per head 65 columns (64 weights + zero col); bias row gets 1.0
        Wg = W[g * HL : (g + 1) * HL]  # [HL, D, A]
        Wa = np.zeros((HL, D, AC), dtype=np.float32)
        Wa[:, :, :A] = Wg
        ba = np.zeros((1, HL * AC), dtype=np.float32)
        bb = bvv[g * HL : (g + 1) * HL]  # [HL, A]
        for h in range(HL):
            ba[0, h * AC : h * AC + A] = bb[h]
            ba[0, h * AC + A] = 1.0
        return (
            np.ascontiguousarray(
                Wa.transpose(1, 0, 2).reshape(D, HL * AC)
            ).astype(np.float16),
            ba.astype(np.float16),
        )

    def pack_b(bvec, g):
        # [H,A] slice -> [128, NPAIR]: column p = concat(b[2p], b[2p+1])
        bg = bvec[g * HL : (g + 1) * HL]
        return np.ascontiguousarray(bg.reshape(NPAIR, 128).T)

    xT_cache = {}
    for b_ in range(B):
        xT_cache[b_] = {
            "q": np.ascontiguousarray(q[b_].T).astype(np.float16),
            "k": np.ascontiguousarray(k[b_].T).astype(np.float16),
            "v": np.ascontiguousarray(v[b_].T).astype(np.float16),
        }

    in_maps = []
    for i in range(NCORES):
        b_, g = i // 2, i % 2
        wv_p, bv_p = pack_wv(Wv, bv, g)
        xc = xT_cache[b_]
        in_maps.append(
            {
                "xq": xc["q"],
                "xk": xc["k"],
                "xv": xc["v"],
                "wq": pack_w(Wq, g),
                "wk": pack_w(Wk, g),
                "wv": wv_p,
                "bq": pack_b(bq, g),
                "bk": pack_b(bk, g),
                "bv": bv_p,
                "ones1": ones1,
            }
        )

    _LAST_IN_MAPS = in_maps
    res = run_bass_kernel_spmd(nc, in_maps, core_ids=list(range(NCORES)))

    out = np.empty((B, S, H * A), dtype=np.float32)
    for i in range(NCORES):
        b_, g = i // 2, i % 2
        r = res.results[i]["out"].reshape(NPAIR, 2, NSQ, AC, 512)
        num = r[:, :, :, 0:A, :]  # [p, hh, sq, a, q']
        den = r[:, :, :, A : A + 1, :]
        y = num / den  # [p, hh, sq, a, q']
        # -> [sq, q', p, hh, a] -> [S, HL*A]
        y = y.transpose(2, 4, 0, 1, 3).reshape(S, HL * A)
        out[b_, :, g * HL * A : (g + 1) * HL * A] = y
    return out


# revision 26
# speedup vs baseline: 1.0501x; 1.0364x over previous
"""Multi-head attention Trainium2 kernel (B=4, S=2048, D=1024, H=16, A=64).

Sharding: 8 cores = batch (4) x head-half (2). Core i handles batch i//2,
heads (i%2)*8 .. (i%2)*8+8. No collectives; host assembles output.

Key design points (v2 — fp16 single-pass matmuls):
  - ALL matmuls run in 16-bit dtypes (1 cyc/row single-pass on the PE at
    2.4 GHz). The previous f32r version was silently lowered to two HW
    passes (fp32_mode=HIGH ~2cyc/row + LOW_HIGH 1cyc/row), doubling PE time.
  - q/k/v arrive HOST-pretransposed [D, S] as fp16 (11-bit mantissa:
    score error sigma ~0.002 absolute, negligible through softmax).
    Projections fp16 x fp16 -> f32 PSUM -> qhT/khT stored fp16.
  - scores^T [Sk, Sq] per head-pair via row-packed K=64 matmul pairs
    (tile_position (0,0)/(64,0)) into one [128,1024] PSUM tile -> single
    1024-col exp on ACT (no max subtraction: |s|max ~48 << 88) with BF16
    output (bf16 shares f32's exponent range; exp(s) can reach e^+48).
  - attn' [65, Sq]: lhsT = vh bf16 [Sk,65] (64 v-cols + ones column),
    rhs = wt bf16. Row 64 accumulates the softmax denominator.
  - NO on-chip normalization: the [65,512] numerator+denominator block is
    copied PSUM->SBUF (DVE) and DMA'd out raw; the host does num/den.
  - Projections are emitted per-pair BETWEEN attention blocks; the Tile
    dataflow scheduler runs them in PE gaps while ACT (exp) is the
    per-block bottleneck, so pair p+1's projections hide under pair p's
    ACT-bound attention. Prologue = proj(pair 0) only.
"""

import sys

sys.path.insert(0, "/opt/trn_rl_repo")

import numpy as np

B, S, D = 4, 2048, 1024
H, A = 16, 64
NCORES = 8
HL = H // 2          # heads per core
NPAIR = HL // 2      # head pairs per core
ND = D // 128        # D chunks
NSQ = S // 512       # Sq chunks
NSK = S // 128       # Sk tiles
AC = A + 1           # vh columns incl. ones column


def _build():
    import concourse.tile as tile
    from concourse import bacc, mybir

    F32 = mybir.dt.float32
    F16 = mybir.dt.float16
    BF16 = mybir.dt.bfloat16
    ADD = mybir.AluOpType.add
    EXP = mybir.ActivationFunctionType.Exp

    nc = bacc.Bacc("TRN2")

    x_d = {}
    for x in ("v", "k", "q"):
        x_d[x] = nc.dram_tensor(f"x{x}", [D, S], F16, kind="ExternalInput").ap()
    wq_d = nc.dram_tensor("wq", [D, HL * A], F16, kind="ExternalInput").ap()
    wk_d = nc.dram_tensor("wk", [D, HL * A], F16, kind="ExternalInput").ap()
    wv_d = nc.dram_tensor("wv", [D, HL * AC], F16, kind="ExternalInput").ap()
    bq_d = nc.dram_tensor("bq", [128, NPAIR], F32, kind="ExternalInput").ap()
    bk_d = nc.dram_tensor("bk", [128, NPAIR], F32, kind="ExternalInput").ap()
    bv_d = nc.dram_tensor("bv", [1, HL * AC], F16, kind="ExternalInput").ap()
    on_d = nc.dram_tensor("ones1", [1, 128], F16, kind="ExternalInput").ap()
    # per (pair, head-in-pair, sq): [65, 512] = numerator rows 0-63, den row 64
    out_d = nc.dram_tensor(
        "out", [NPAIR * 2 * NSQ * AC, 512], F32, kind="ExternalOutput"
    ).ap()

    with tile.TileContext(nc) as tc:
        with (
            tc.tile_pool(name="consts", bufs=1) as consts,
            tc.tile_pool(name="stage", bufs=1) as stage,
            tc.tile_pool(name="persist", bufs=1) as persist,
            tc.tile_pool(name="work", bufs=1) as work,
            tc.tile_pool(name="ps", bufs=1, space="PSUM") as ps,
        ):
            ones1 = consts.tile([1, 128], F16, tag="ones1")
            bq_sb = consts.tile([128, NPAIR], F32, tag="bq")
            bk_sb = consts.tile([128, NPAIR], F32, tag="bk")
            bv_sb = consts.tile([1, HL * AC], F16, tag="bv")
            nc.sync.dma_start(ones1, on_d)
            nc.sync.dma_start(bq_sb, bq_d)
            nc.sync.dma_start(bk_sb, bk_d)
            nc.sync.dma_start(bv_sb, bv_d)

            # Input staging: one [128, S] tile per D-chunk. DMA bandwidth is
            # serialized (~400GB/s aggregate), so order = arrival time.
            # Weights go just-in-time before their first consumer; the first
            # k/q chunks are 512-col so kproj(0,h0)/qproj(0,h0) — the
            # exp-critical path — start as early as possible.
            xt = {}
            for x in ("v", "k", "q"):
                xt[x] = [
                    stage.tile([128, S], F16, tag=f"x{x}{d}", name=f"x{x}{d}")
                    for d in range(ND)
                ]
            wk_sb = stage.tile([128, ND, HL * A], F16, tag="wk", name="wk_sb")
            wq_sb = stage.tile([128, ND, HL * A], F16, tag="wq", name="wq_sb")
            wv_sb = stage.tile([128, ND, HL * AC], F16, tag="wv", name="wv_sb")

            def xdma(x, c0, w):
                for d in range(ND):
                    nc.sync.dma_start(
                        xt[x][d][:, c0 : c0 + w],
                        x_d[x][d * 128 : (d + 1) * 128, c0 : c0 + w],
                    )

            nc.sync.dma_start(wk_sb, wk_d.rearrange("(c p) n -> p c n", p=128))
            xdma("k", 0, 1024)
            nc.sync.dma_start(wq_sb, wq_d.rearrange("(c p) n -> p c n", p=128))
            xdma("q", 0, 1024)
            nc.sync.dma_start(wv_sb, wv_d.rearrange("(c p) n -> p c n", p=128))
            xdma("v", 0, 1024)
            xdma("k", 1024, 1024)
            xdma("v", 1024, 1024)
            xdma("q", 1024, 1024)

            qhT = [
                persist.tile([128, S], F16, tag=f"qhT{p}", name=f"qhT{p}")
                for p in range(NPAIR)
            ]
            khT = [
                persist.tile([128, S], F16, tag=f"khT{p}", name=f"khT{p}")
                for p in range(NPAIR)
            ]
            vh = persist.tile([128, HL, NSK, AC], BF16, tag="vh")

            def vproj(p, m0=0, m1=NSK):
                # vh[:, 2p:2p+2, m, :]: [Sk, 2*65] with ones column via
                # K=1 bias matmul (weights col 64 of each head = 0, bias = 1)
                cw = 2 * AC
                for m in range(m0, m1):
                    pv = ps.tile([128, cw], F32, tag="pp", name="pv", bufs=2)
                    for d in range(ND):
                        nc.tensor.matmul(
                            pv,
                            xt["v"][d][:, m * 128 : (m + 1) * 128],
                            wv_sb[:, d, p * cw : (p + 1) * cw],
                            start=(d == 0),
                            stop=False,
                        )
                    nc.tensor.matmul(
                        pv, ones1, bv_sb[:, p * cw : (p + 1) * cw],
                        start=False, stop=True,
                    )
                    nc.vector.tensor_copy(
                        vh[:, 2 * p : 2 * p + 2, m, :],
                        pv.rearrange("p (h c) -> p h c", h=2),
                    )

            def kqproj(x, p, w_sb, bias_sb, dst, half):
                # dst[p] [128 (2 heads), S] fp16 = W_pair^T @ x^T + bias
                col = half * 512
                pph = ps.tile([128, 512], F32, tag="pp", name="pph", bufs=2)
                for d in range(ND):
                    nc.tensor.matmul(
                        pph,
                        w_sb[:, d, p * 128 : (p + 1) * 128],
                        xt[x][d][:, col : col + 512],
                        start=(d == 0),
                        stop=(d == ND - 1),
                    )
                nc.vector.tensor_scalar(
                    dst[p][:, col : col + 512],
                    pph,
                    bias_sb[:, p : p + 1],
                    None,
                    ADD,
                )

            # Global software pipeline across ALL (p, sq, sk): scores/exp
            # lead attention by 2 steps and the pipeline never drains at
            # block boundaries, so the exp stream on ACT stays contiguous.
            Pstate = {}

            def emit_scores(p, sq, sk):
                Sc = ps.tile([128, 1024], F32, tag="sc", name="Sc", bufs=2)
                nc.tensor.matmul(
                    Sc[:, 0:512],
                    khT[p][0:64, sk * 128 : (sk + 1) * 128],
                    qhT[p][0:64, sq * 512 : (sq + 1) * 512],
                    start=True,
                    stop=True,
                    tile_position=(0, 0),
                )
                nc.tensor.matmul(
                    Sc[:, 512:1024],
                    khT[p][64:128, sk * 128 : (sk + 1) * 128],
                    qhT[p][64:128, sq * 512 : (sq + 1) * 512],
                    start=True,
                    stop=True,
                    tile_position=(64, 0),
                )
                wt = work.tile([128, 1024], BF16, tag="wt", name="wt", bufs=4)
                nc.scalar.activation(wt, Sc, EXP)
                return wt

            def emit_attn(p, sq, k0, wt):
                h0, h1 = 2 * p, 2 * p + 1
                if k0 == 0:
                    P0 = ps.tile([AC, 512], F32, tag="att", name="P0", bufs=2)
                    P1 = ps.tile([AC, 512], F32, tag="att", name="P1", bufs=2)
                    Pstate[(p, sq)] = (P0, P1)
                P0, P1 = Pstate[(p, sq)]
                st = k0 == 0
                sp = k0 == NSK - 1
                nc.tensor.matmul(
                    P0, vh[:, h0, k0, :], wt[:, 0:512], start=st, stop=sp,
                )
                nc.tensor.matmul(
                    P1, vh[:, h1, k0, :], wt[:, 512:1024], start=st, stop=sp,
                )
                if sp:
                    for hh, P in ((0, P0), (1, P1)):
                        atts = work.tile(
                            [AC, 512], F32, tag="atts", name="atts", bufs=4
                        )
                        nc.vector.tensor_copy(atts, P)
                        blk = (p * 2 + hh) * NSQ + sq
                        nc.sync.dma_start(
                            out_d[blk * AC : (blk + 1) * AC, :], atts
                        )
                    del Pstate[(p, sq)]

            # Emission order == scheduler priority, and dep tracking is
            # program-order-causal, so each projection chunk must be emitted
            # before its first reader but as CLOSE to it as possible (earlier
            # emission = higher priority = lumps at pair boundaries). The
            # fill lists below spread next-pair projections through the
            # current pair's ACT-bound blocks, meeting every deadline:
            #   kp(p,0/1), qp(p,0) before block(p,0); kp(p,2/3) before its
            #   sk=8 scores; qp(p,h) before block(p,h); v(p,m) before the
            #   attn read at sk=m+2 of block(p,0).
            def kp(p, h):
                return lambda: kqproj("k", p, wk_sb, bk_sb, khT, h)

            def qp(p, h):
                return lambda: kqproj("q", p, wq_sb, bq_sb, qhT, h)

            def vp(p, m):
                return lambda: vproj(p, m, m + 1)

            kp(0, 0)()
            qp(0, 0)()
            kp(0, 1)()
            vproj(0)

            fills = {
                (0, 0): [kp(0, 2), kp(0, 3), qp(0, 1)],
                (0, 1): [qp(0, 2), qp(0, 3), kp(1, 0), qp(1, 0)],
                (0, 2): [kp(1, 1), kp(1, 2)] + [vp(1, m) for m in range(4)],
                (0, 3): [kp(1, 3), qp(1, 1)] + [vp(1, m) for m in range(4, 10)],
                (1, 0): [vp(1, m) for m in range(10, 16)] + [qp(1, 2)],
                (1, 1): [qp(1, 3), kp(2, 0), qp(2, 0), kp(2, 1)],
                (1, 2): [kp(2, 2), kp(2, 3)] + [vp(2, m) for m in range(4)],
                (1, 3): [qp(2, 1)] + [vp(2, m) for m in range(4, 10)],
                (2, 0): [vp(2, m) for m in range(10, 16)] + [qp(2, 2)],
                (2, 1): [qp(2, 3), kp(3, 0), qp(3, 0), kp(3, 1)],
                (2, 2): [kp(3, 2), kp(3, 3)] + [vp(3, m) for m in range(4)],
                (2, 3): [qp(3, 1)] + [vp(3, m) for m in range(4, 10)],
                (3, 0): [vp(3, m) for m in range(10, 16)] + [qp(3, 2)],
                (3, 1): [qp(3, 3)],
            }
            for p in range(NPAIR):
                for sq in range(NSQ):
                    fl = list(fills.get((p, sq), ()))
                    pend = []
                    for sk in range(NSK + 2):
                        if sk >= 2 and fl:
                            fl.pop(0)()
                        if sk < NSK:
                            wt = emit_scores(p, sq, sk)
                            pend.append((p, sq, sk, wt))
                        if sk >= 2:
                            emit_attn(*pend.pop(0))

    nc.compile()
    return nc


_NC_CACHE = None
_LAST_IN_MAPS = None


def kernel(**inputs: np.ndarray) -> np.ndarray:
    global _NC_CACHE, _LAST_IN_MAPS

    from concourse.bass_utils import run_bass_kernel_spmd

    q = np.ascontiguousarray(inputs["q"], dtype=np.float32)
    k = np.ascontiguousarray(inputs["k"], dtype=np.float32)
    v = np.ascontiguousarray(inputs["v"], dtype=np.float32)
    Wq = np.asarray(inputs["Wq"], dtype=np.float32)
    Wk = np.asarray(inputs["Wk"], dtype=np.float32)
    Wv = np.asarray(inputs["Wv"], dtype=np.float32)
    bq = np.asarray(inputs["bq"], dtype=np.float32)
    bk = np.asarray(inputs["bk"], dtype=np.float32)
    bv = np.asarray(inputs["bv"], dtype=np.float32)

    if _NC_CACHE is None:
        _NC_CACHE = _build()
    nc = _NC_CACHE

    ones1 = np.ones((1, 128), dtype=np.float16)

    def pack_w(W, g):
        # [H,D,A] slice -> [D, HL*A], heads side by side
        return np.ascontiguousarray(
            W[g * HL : (g + 1) * HL].transpose(1, 0, 2).reshape(D, HL * A)
        ).astype(np.float16)

    def pack_wv(W, bvv, g):
        # augmented: per head 65 columns (64 weights + zero col); bias row gets 1.0
        Wg = W[g * HL : (g + 1) * HL]  # [HL, D, A]
        Wa = np.zeros((HL, D, AC), dtype=np.float32)
        Wa[:, :, :A] = Wg
        ba = np.zeros((1, HL * AC), dtype=np.float32)
        bb = bvv[g * HL : (g + 1) * HL]  # [HL, A]
        for h in range(HL):
            ba[0, h * AC : h * AC + A] = bb[h]
            ba[0, h * AC + A] = 1.0
        return (
            np.ascontiguousarray(
                Wa.transpose(1, 0, 2).reshape(D, HL * AC)
            ).astype(np.float16),
            ba.astype(np.float16),
        )

    def pack_b(bvec, g):
        # [H,A] slice -> [128, NPAIR]: column p = concat(b[2p], b[2p+1])
        bg = bvec[g * HL : (g + 1) * HL]
        return np.ascontiguousarray(bg.reshape(NPAIR, 128).T)

    xT_cache = {}
    for b_ in range(B):
        xT_cache[b_] = {
            "q": np.ascontiguousarray(q[b_].T).astype(np.float16),
            "k": np.ascontiguousarray(k[b_].T).astype(np.float16),
            "v": np.ascontiguousarray(v[b_].T).astype(np.float16),
        }

    in_maps = []
    for i in range(NCORES):
        b_, g = i // 2, i % 2
        wv_p, bv_p = pack_wv(Wv, bv, g)
        xc = xT_cache[b_]
        in_maps.append(
            {
                "xq": xc["q"],
                "xk": xc["k"],
                "xv": xc["v"],
                "wq": pack_w(Wq, g),
                "wk": pack_w(Wk, g),
                "wv": wv_p,
                "bq": pack_b(bq, g),
                "bk": pack_b(bk, g),
                "bv": bv_p,
                "ones1": ones1,
            }
        )

    _LAST_IN_MAPS = in_maps
    res = run_bass_kernel_spmd(nc, in_maps, core_ids=list(range(NCORES)))

    out = np.empty((B, S, H * A), dtype=np.float32)
    for i in range(NCORES):
        b_, g = i // 2, i % 2
        r = res.results[i]["out"].reshape(NPAIR, 2, NSQ, AC, 512)
        num = r[:, :, :, 0:A, :]  # [p, hh, sq, a, q']
        den = r[:, :, :, A : A + 1, :]
        y = num / den  # [p, hh, sq, a, q']
        # -> [sq, q', p, hh, a] -> [S, HL*A]
        y = y.transpose(2, 4, 0, 1, 3).reshape(S, HL * A)
        out[b_, :, g * HL * A : (g + 1) * HL * A] = y
    return out
